# revision 10
# baseline (speedup 1.0000x reference)
"""Multi-head cross-attention Trainium2 kernel (phased bf16 pipeline).

Full-input contract: kernel(**inputs) takes the complete tensors and returns
the complete output. Internally shards over 8 NeuronCores as
(batch x head-group): core c handles batch c//4 and heads [4*(c%4), 4*(c%4)+4).
Each core computes its partial output  ctx_g @ Wo_g  for its batch; the host
sums the 4 head-group partials per batch and adds bo_eff = bo + bv @ Wo.

Masked keys (key_mask == 0) contribute exactly zero probability, so the host
compacts key/value to the unmasked rows (padded up to a multiple of 128 with
-1e9 score bias).

v2 design vs v1: the 4 per-core heads are processed in TWO PHASES of a head
pair (phase t covers heads {2t, 2t+1} = weight tile t). Halving the number of
concurrently-accumulating heads frees PSUM banks, letting each softmax exp
cover a full [128 sk, 1024 sq] tile (one ACT instruction per (head, skt)
instead of two): ACT's fixed ~370ns SBUF-access overhead per instruction is
the co-bottleneck with the PE, and this halves its count. Phase 1's epilogue
(reciprocal, ctx normalize, ctx transpose) runs DURING phase 2's attention on
the otherwise-idle DVE/DMA engines (transposes via the DMA XBAR), so only
phase 2's epilogue remains on the critical path. PSUM budget (8 banks):
pp 2x[128,1024] scores tiles (4) + pa 2x per-head ctx (2) + pd denominators
(1) + pj projection scratch (1).

Per-core pipeline per phase t, step (skt, l), h = 2t + l:
  S^T[sk 128, sq 1024] = kT_h x qT_h  (two 512-wide matmuls, one 2-bank tile)
  es = exp(scale*S^T + maskbias)      (ONE [128,1024] ACT instruction)
  ctx/denominator accumulate in PSUM over skt with es tiles as lhsT.
Q/K/V projections ride a paced work queue interleaved between steps, as in
v1; kT t=1 and qT t=1 are projected during/before phase 2 from SBUF-resident
inputs. Output is fp16 (halves the out-DMA); host accumulates in fp32.
"""

import numpy as np
import ml_dtypes

B, SQ, SK, IN = 2, 1024, 4096, 1024
H_TOT, D, HPC = 16, 64, 4
DH = HPC * D  # 256, per-core head-dim slice
NCORES = 8
BF_NP = ml_dtypes.bfloat16

_CACHE = {}

# scheduling knobs
CFG = {"pp": 2, "ses": 22, "sin": 12, "sout": 5, "warm": 13, "delay": 3,
       "npre": 12, "fill": 2, "k1b0": 4, "actq": 2, "tpool": 0, "pre0": 12, "pstride": 3,
       "edma": 1, "skew": 2, "dualq": 0, "split0": 1}


def _blocks_of(width, step=512):
    out, off = [], 0
    while off < width:
        w = min(step, width - off)
        out.append((off, w))
        off += w
    return out


def _build(skp):
    import concourse.tile as tile
    from concourse import bacc, mybir

    FP = mybir.dt.float32
    F16 = mybir.dt.float16
    BF = mybir.dt.bfloat16
    AF = mybir.ActivationFunctionType

    nc = bacc.Bacc("TRN2", target_bir_lowering=False, debug=False)

    qT_d = nc.dram_tensor("qT", [IN, SQ], BF, kind="ExternalInput").ap()
    kT_d = nc.dram_tensor("kT", [IN, skp], BF, kind="ExternalInput").ap()
    vT_d = nc.dram_tensor("vT", [IN, skp], BF, kind="ExternalInput").ap()
    wq_d = nc.dram_tensor("wq", [IN, DH], BF, kind="ExternalInput").ap()
    wk_d = nc.dram_tensor("wk", [IN, DH], BF, kind="ExternalInput").ap()
    wv_d = nc.dram_tensor("wv", [IN, DH], BF, kind="ExternalInput").ap()
    wo_d = nc.dram_tensor("wo", [DH, SQ], BF, kind="ExternalInput").ap()
    bqk_d = nc.dram_tensor("bqk", [128, 4], FP, kind="ExternalInput").ap()
    mb_d = nc.dram_tensor("mb", [128, skp // 128], FP, kind="ExternalInput").ap()
    ones_d = nc.dram_tensor("ones", [128, 1], BF, kind="ExternalInput").ap()
    idn_d = nc.dram_tensor("idn", [128, 128], BF, kind="ExternalInput").ap()
    out_d = nc.dram_tensor("out", [SQ, SQ], F16, kind="ExternalOutput").ap()

    NSKT = skp // 128          # sk tiles of 128
    NKC = IN // 128            # 8 contraction chunks
    NSQT = SQ // 128           # 8 sq tiles
    SCALE = 1.0 / float(np.sqrt(D))
    blocks = _blocks_of(skp)
    P1 = 2 * NSKT              # steps in phase 1

    with tile.TileContext(nc) as tc:
        cpool_cm = tc.tile_pool(name="const", bufs=1)
        cpool = cpool_cm.__enter__()
        wq_sb = cpool.tile([128, NKC, DH], BF, name="wq_sb")
        wk_sb = cpool.tile([128, NKC, DH], BF, name="wk_sb")
        wv_sb = cpool.tile([128, NKC, DH], BF, name="wv_sb")
        wo_sb = cpool.tile([128, 2, SQ], BF, name="wo_sb")
        bqk_sb = cpool.tile([128, 4], FP, name="bqk_sb")
        mb_sb = cpool.tile([128, NSKT], FP, name="mb_sb")
        ones_sb = cpool.tile([128, 1], BF, name="ones_sb")
        idn_sb = cpool.tile([128, 128], BF, name="idn_sb")
        qT_sb = cpool.tile([128, 2, SQ], BF, name="qT_sb")
        kT_sb = cpool.tile([128, 2, skp], BF, name="kT_sb")
        v_sb = cpool.tile([128, NSKT, DH], BF, name="v_sb")
        junk_sb = cpool.tile([64, 512], BF, name="junk_sb")
        ctx_sb = cpool.tile([128, 2, NSQT, 128], BF, name="ctx_sb")
        ctxT_sb = cpool.tile([128, 2, SQ], BF, name="ctxT_sb")
        rec_sb = cpool.tile([128, 2, 16], FP, name="rec_sb")

        def dma(dst, src):
            nc.sync.dma_start(out=dst, in_=src)

        def dma_act(dst, src):
            nc.scalar.dma_start(out=dst, in_=src)

        # critical-path loads first; tiny constants ride behind qin0.
        dma(wq_sb[:], wq_d.rearrange("(kc p) n -> p kc n", p=128))
        nc.vector.memset(junk_sb[:], 0.0)

        with tc.tile_pool(name="sin", bufs=CFG["sin"]) as sin, \
             tc.tile_pool(name="ses", bufs=CFG["ses"]) as ses, \
             tc.tile_pool(name="sout", bufs=CFG["sout"]) as sout, \
             tc.tile_pool(name="pp", bufs=CFG["pp"], space="PSUM") as pp, \
             tc.tile_pool(name="pa", bufs=2, space="PSUM") as pa, \
             tc.tile_pool(name="pd", bufs=1, space="PSUM") as pd, \
             tc.tile_pool(name="pj", bufs=1, space="PSUM") as pj:

            state = {}

            def load_block(x_d, off, w, name):
                xin = sin.tile([128, NKC, 512], BF, tag="sin", name=name)
                dma(xin[:, :, 0:w],
                    x_d.rearrange("(kc p) n -> p kc n", p=128)[:, :, off:off + w])
                return xin

            def qk_proj(w_sb, xin, dst_sb, bias_col0, off, w, t, src=0):
                # full 8-chunk projection of one <=512-wide block, tile t
                ps = pp.tile([128, 512], FP, tag="pp", name="ps")
                for kc in range(NKC):
                    nc.tensor.matmul(
                        ps[:, 0:w],
                        lhsT=w_sb[:, kc, t * 128:(t + 1) * 128],
                        rhs=xin[:, kc, src:src + w],
                        start=(kc == 0), stop=(kc == NKC - 1))
                with nc.allow_low_precision(reason="bf16 storage"):
                    nc.vector.tensor_scalar_add(
                        dst_sb[:, t, off:off + w], ps[:, 0:w],
                        bqk_sb[:, bias_col0 + t:bias_col0 + t + 1])

            def qk_part(w_sb, xin, dst_sb, bias_col0, off, w, t, part, key,
                        src=0):
                # quarter-sized projection work item (2 of 8 kc chunks)
                if part == 0:
                    state[key] = pj.tile([128, 512], FP, tag="pj", name="ps")
                ps = state[key]
                for kc in (2 * part, 2 * part + 1):
                    nc.tensor.matmul(
                        ps[:, 0:w],
                        lhsT=w_sb[:, kc, t * 128:(t + 1) * 128],
                        rhs=xin[:, kc, src:src + w],
                        start=(kc == 0), stop=(kc == NKC - 1))
                if part == 3:
                    with nc.allow_low_precision(reason="bf16 storage"):
                        nc.vector.tensor_scalar_add(
                            dst_sb[:, t, off:off + w], ps[:, 0:w],
                            bqk_sb[:, bias_col0 + t:bias_col0 + t + 1])

            def v_part(xin, off, j, part, key):
                # half-sized V work item (4 of 8 kc chunks) for sk tile j
                if part == 0:
                    state[key] = pj.tile([128, DH], FP, tag="pj", name="psv")
                ps = state[key]
                for kc in range(4 * part, 4 * part + 4):
                    nc.tensor.matmul(
                        ps[:, :],
                        lhsT=xin[:, kc, j * 128:(j + 1) * 128],
                        rhs=wv_sb[:, kc, :],
                        start=(kc == 0), stop=(kc == NKC - 1))
                if part == 1:
                    with nc.allow_low_precision(reason="bf16 storage"):
                        nc.vector.tensor_copy(
                            v_sb[:, off // 128 + j, :], ps[:, :])

            def scores_exp(t, skt, l):
                r0 = 64 * l
                ps_s = pp.tile([128, 1024], FP, tag="pp", name="ps_s")
                for half in (0, 1):
                    nc.tensor.matmul(
                        ps_s[:, half * 512:(half + 1) * 512],
                        lhsT=kT_sb[r0:r0 + 64, t, skt * 128:(skt + 1) * 128],
                        rhs=qT_sb[r0:r0 + 64, t, half * 512:(half + 1) * 512],
                        start=True, stop=True)
                es = ses.tile([128, 1024], BF, tag="es", name="es")
                with nc.allow_low_precision(reason="bf16 storage"):
                    nc.scalar.activation(
                        es[:, :], ps_s[:, :], AF.Exp,
                        bias=mb_sb[:, skt:skt + 1], scale=SCALE)
                return es

            def ctx_acc(es, t, skt, l):
                # pa bank = head (pool rotates phase-to-phase); pd bank is
                # per-phase (one accumulation group per phase, bufs=1 pool).
                h = 2 * t + l
                if skt == 0:
                    state[("pa", h)] = pa.tile([128, NSQT, 64], FP,
                                               tag="pa", name=f"pa{h}")
                    if l == 0:
                        state[("pd", t)] = pd.tile([128, 16], FP,
                                                   tag="pd", name=f"pd{t}")
                pa_t = state[("pa", h)]
                pd_t = state[("pd", t)]
                for g in range(NSQT):
                    nc.tensor.matmul(
                        pa_t[:, g, :],
                        lhsT=es[:, g * 128:(g + 1) * 128],
                        rhs=v_sb[:, skt, h * 64:(h + 1) * 64],
                        start=(skt == 0 and g == 0),
                        stop=(skt == NSKT - 1 and g == NSQT - 1))
                    nc.tensor.matmul(
                        pd_t[:, l * 8 + g:l * 8 + g + 1],
                        lhsT=es[:, g * 128:(g + 1) * 128],
                        rhs=ones_sb[:, 0:1],
                        start=(skt == 0 and l == 0 and g == 0),
                        stop=(skt == NSKT - 1 and l == 1 and g == NSQT - 1))

            # ---- phase-1 epilogue (runs during phase 2) ----
            def recip(t):
                nc.vector.reciprocal(rec_sb[:, t, :], state[("pd", t)][:, :])

            def norm_one(t, l, sqt, eng):
                h = 2 * t + l
                pa_t = state[("pa", h)]
                c = l * 8 + sqt
                with nc.allow_low_precision(reason="bf16 storage"):
                    if eng == "dve":
                        nc.vector.tensor_scalar_mul(
                            ctx_sb[:, t, sqt, l * 64:(l + 1) * 64],
                            pa_t[:, sqt, :], rec_sb[:, t, c:c + 1])
                    elif eng == "pool":
                        nc.gpsimd.tensor_scalar_mul(
                            ctx_sb[:, t, sqt, l * 64:(l + 1) * 64],
                            pa_t[:, sqt, :], rec_sb[:, t, c:c + 1])
                    else:
                        nc.scalar.activation(
                            ctx_sb[:, t, sqt, l * 64:(l + 1) * 64],
                            pa_t[:, sqt, :], AF.Copy,
                            scale=rec_sb[:, t, c:c + 1])

            def xbar_transpose(t, sqt):
                # ctx [sq 128, dh 128] -> ctxT [dh 128, sq 128] via DMA XBAR
                nc.sync.dma_start(
                    out=ctxT_sb[:, t, sqt * 128:(sqt + 1) * 128],
                    in_=ctx_sb[:, t, sqt, :], transpose=True)

            # ---- prologue ----
            # DMA order puts the full qT first: all four Q projections then
            # run back-to-back on the PE while kT/vT stream in behind, so the
            # PE never waits for data once the junk warmup ends.
            w0 = blocks[0][1]
            if CFG.get("dualq", 1):
                # split the serial prologue load stream over both hardware
                # DGE queues (SP + ACT): per-DMA issue/HWDGE overheads hide
                # behind the other queue's transfer, so the DMA engines
                # stream back-to-back through the DMA-bound start.
                qin0 = sin.tile([128, NKC, 512], BF, tag="sin", name="qin0")
                dma_act(qin0[:, :, 0:512],
                        qT_d.rearrange("(kc p) n -> p kc n", p=128)[:, :, 0:512])
                dma_act(bqk_sb[:], bqk_d[:, :])
                dma_act(mb_sb[:], mb_d[:, :])
                qin1 = load_block(qT_d, 512, 512, "qin1")
                dma_act(wk_sb[:], wk_d.rearrange("(kc p) n -> p kc n", p=128))
                kin0 = load_block(kT_d, 0, w0, "kin0")
                dma_act(wv_sb[:], wv_d.rearrange("(kc p) n -> p kc n", p=128))
                vin0 = load_block(vT_d, 0, w0, "vin0")
                dma_act(ones_sb[:], ones_d[:, :])
                dma_act(idn_sb[:], idn_d[:, :])
            else:
                qin0 = load_block(qT_d, 0, 512, "qin0")
                dma(bqk_sb[:], bqk_d[:, :])
                dma(mb_sb[:], mb_d[:, :])
                qin1 = load_block(qT_d, 512, 512, "qin1")
                dma(wk_sb[:], wk_d.rearrange("(kc p) n -> p kc n", p=128))
                kin0 = load_block(kT_d, 0, w0, "kin0")
                dma(wv_sb[:], wv_d.rearrange("(kc p) n -> p kc n", p=128))
                vin0 = load_block(vT_d, 0, w0, "vin0")
                dma(ones_sb[:], ones_d[:, :])
                dma(idn_sb[:], idn_d[:, :])

            def junk_mm(n):
                for i in range(n):
                    wm = pj.tile([64, 512], FP, tag="pj", name="wm")
                    nc.tensor.matmul(wm[:, :], lhsT=junk_sb[:, 0:64],
                                     rhs=junk_sb[:, :], start=True, stop=True)

            junk_mm(CFG.get("warm", 10))

            # inline: qT t0+t1 (both blocks), junk filler, kT t0 block 0
            qk_proj(wq_sb, qin0, qT_sb, 0, 0, 512, 0)
            qk_proj(wq_sb, qin0, qT_sb, 0, 0, 512, 1)
            qk_proj(wq_sb, qin1, qT_sb, 0, 512, 512, 0)
            qk_proj(wq_sb, qin1, qT_sb, 0, 512, 512, 1)
            junk_mm(CFG.get("fill", 3))
            if CFG.get("split0", 1) and w0 > 256:
                qk_proj(wk_sb, kin0, kT_sb, 2, 0, 256, 0)
                qk_proj(wk_sb, kin0, kT_sb, 2, 256, w0 - 256, 0, src=256)
            else:
                qk_proj(wk_sb, kin0, kT_sb, 2, 0, w0, 0)

            # ---- paced work queue ----
            work = []
            xins = {("k", 0): kin0, ("v", 0): vin0}

            def qkw(due, w_sb, xin_key, dst, b0, off, w, t, key, src=0):
                for part in range(4):
                    work.append((due[part],
                                 lambda part=part: qk_part(
                                     w_sb, xins[xin_key], dst, b0, off, w, t,
                                     part, key, src)))

            def vw(due, bi, off, j, key):
                for part in range(2):
                    work.append((due[part],
                                 lambda part=part: v_part(
                                     xins[("v", bi)], off, j, part, key)))

            DELAY = CFG.get("delay", 3)
            NPRE = min(CFG.get("npre", 8), 2 * NSKT - 2)

            # Build the slot list first so work dues can be stated in slot
            # indices. Prefetched phase-2 steps are spread from slot PRE0
            # every PSTRIDE slots: their exps fill ACT's idle cycles across
            # all of the (PE/DMA-bound) phase 1, including the early
            # DMA-starved window.
            PRE0 = CFG.get("pre0", 8)
            PSTRIDE = CFG.get("pstride", 3)
            p1_steps = [(0, skt, l) for skt in range(NSKT) for l in (0, 1)]
            p2_steps = [(1, skt, l) for skt in range(NSKT) for l in (0, 1)]
            pre, native2 = p2_steps[:NPRE], p2_steps[NPRE:]
            if CFG.get("ppair", 0):
                pre_pos = {PRE0 + 6 * k + j
                           for k in range((NPRE + 1) // 2) for j in (0, 1)}
            else:
                pre_pos = {PRE0 + PSTRIDE * k for k in range(NPRE)}
            slots, slot_of_p1 = [], []
            n1 = pi = i = 0
            while n1 < len(p1_steps) or pi < len(pre):
                if pi < len(pre) and (i in pre_pos or n1 >= len(p1_steps)):
                    slots.append(pre[pi])
                    pi += 1
                else:
                    slot_of_p1.append(i)
                    slots.append(p1_steps[n1])
                    n1 += 1
                i += 1
            P1S = len(slots)  # slot where native phase 2 begins
            slots.extend(native2)

            def s1(n):  # slot index of phase-1 step n
                return slot_of_p1[min(n, len(slot_of_p1) - 1)]

            # V tiles of block 0 — dues no earlier than the vin0 landing,
            # or the open pj partial would block later-ready work (priority
            # inversion on the single pj bank).
            for j in range(w0 // 128):
                d = 3 + 2 * j
                vw([d, d + 1], 0, 0, j, f"v0{j}")
            # kT t0 blocks 1.. + V blocks 1..
            for bi in range(1, len(blocks)):
                off, w = blocks[bi]

                def mk_dma(off=off, w=w, bi=bi):
                    xins[("k", bi)] = load_block(kT_d, off, w, f"kin{bi}")
                    xins[("v", bi)] = load_block(vT_d, off, w, f"vin{bi}")
                work.append((max(0, 2 * bi - 1 if CFG.get("edma", 1)
                              else s1(8 * bi) - 14), mk_dma))
                d0 = s1(8 * bi) - 7
                qkw([d0, d0, d0 + 1, d0 + 1], wk_sb, ("k", bi), kT_sb, 2,
                    off, w, 0, ("k0", bi))
                for j in range(w // 128):
                    d = s1(8 * bi + 2 * j) + DELAY - 2
                    vw([d, d + 1], bi, off, j, ("v", bi, j))
            # kT t1: block 0 early (feeds the prefetched phase-2 steps),
            # the rest timed to native phase 2
            kd = CFG.get("k1b0", 5)
            qkw([kd, kd, kd + 1, kd + 1], wk_sb, ("k", 0), kT_sb, 2, 0, w0,
                1, "k1b0")
            for bi in range(1, len(blocks)):
                off, w = blocks[bi]
                # native phase 2 reaches skt 4*bi at slot P1S + 2*(4*bi - 4)
                d0 = max(P1S - 10, P1S + 8 * bi - 15)
                qkw([d0, d0, d0 + 1, d0 + 1], wk_sb, ("k", bi), kT_sb, 2,
                    off, w, 1, ("k1", bi))
            # wo load late (needed only at the tail)
            work.append((P1S - 6, lambda: dma(
                wo_sb[:], wo_d.rearrange("(t p) n -> p t n", p=128))))
            # phase-1 epilogue rides phase 2: reciprocal, normalizes (DVE),
            # XBAR transposes (DMA) — no PE/ACT time stolen. Norm dues are
            # compressed (4/slot) so the pa-bank WAR that gates phase-2 ctx
            # clears before the first phase-2 ctx_acc issues.
            work.append((P1S, lambda: recip(0)))
            for sqt in range(NSQT):
                for l in (0, 1):
                    work.append((P1S + 1 + sqt // 4,
                                 lambda l=l, sqt=sqt: norm_one(0, l, sqt,
                                                               "dve")))
                work.append((P1S + 2 + sqt // 2,
                             lambda sqt=sqt: xbar_transpose(0, sqt)))

            work.sort(key=lambda x: x[0])      # stable: preserves dep order
            work.reverse()  # pop from end

            # ---- main attention loop ----
            # Prefetched phase-2 ctx_accs are held until native phase 2 is
            # underway so the pa-bank WAR on phase 1's normalize cannot
            # head-block the PE queue.
            pend1, pend2 = [], []
            for i, (t, skt, l) in enumerate(slots):
                es = scores_exp(t, skt, l)
                (pend1 if t == 0 else pend2).append((es, t, skt, l))
                while len(pend1) > DELAY:
                    ctx_acc(*pend1.pop(0))
                if i >= P1S:
                    # native phase 2: finish all phase-1 ctx (normalize work
                    # items at P1S+1.. depend on the banks being closed)
                    while pend1:
                        ctx_acc(*pend1.pop(0))
                if i >= P1S + 4:
                    # drain the prefetch backlog two per slot (three near the
                    # end, so the final post-loop ctx burst that gates the
                    # reciprocal/normalize chain is as small as possible)
                    kmax = 2 if i < len(slots) - 5 else 3
                    floor = DELAY if i < len(slots) - 2 else 1
                    k = 0
                    while len(pend2) > floor and k < kmax:
                        ctx_acc(*pend2.pop(0))
                        k += 1
                while work and work[-1][0] <= i:
                    work.pop()[1]()
            while work:
                work.pop()[1]()
            while pend1:
                ctx_acc(*pend1.pop(0))
            while pend2:
                ctx_acc(*pend2.pop(0))

            # ---- final tail: phase-2 epilogue + output projection ----
            # Four independent PSUM chains (transpose via pp; out-proj lo0
            # via pa's two banks, lo1 alternating pd/pj) and per-sqt
            # interleaved program order with a one-sqt skew, so the chains
            # pipeline instead of serializing on in-order engine queues.
            recip(1)

            def tail_mms(sqt):
                ps_a = pa.tile([128, 512], FP, tag="pa", name="ps_a")
                ps_b = (pd if sqt % 2 == 0 else pj).tile(
                    [128, 512], FP, tag="pd" if sqt % 2 == 0 else "pj",
                    name="ps_b")
                for lo, ps in ((0, ps_a), (512, ps_b)):
                    for t in (0, 1):
                        nc.tensor.matmul(
                            ps[:, :],
                            lhsT=ctxT_sb[:, t, sqt * 128:(sqt + 1) * 128],
                            rhs=wo_sb[:, t, lo:lo + 512],
                            start=(t == 0), stop=(t == 1))
                o_sb = sout.tile([128, SQ], F16, tag="o", name="o_sb")
                # split halves: each half's DMA departs right after its own
                # copy, shortening the drain after the last copy.
                with nc.allow_low_precision(reason="fp16 storage"):
                    nc.vector.tensor_copy(o_sb[:, 0:512], ps_a[:, :])
                dma(out_d[sqt * 128:(sqt + 1) * 128, 0:512], o_sb[:, 0:512])
                with nc.allow_low_precision(reason="fp16 storage"):
                    nc.scalar.copy(o_sb[:, 512:1024], ps_b[:, :])
                aq = CFG.get("actq", 1)
                if aq == 1:
                    nc.scalar.dma_start(
                        out=out_d[sqt * 128:(sqt + 1) * 128, 512:1024],
                        in_=o_sb[:, 512:1024])
                elif aq == 2:
                    nc.gpsimd.dma_start(
                        out=out_d[sqt * 128:(sqt + 1) * 128, 512:1024],
                        in_=o_sb[:, 512:1024])
                else:
                    dma(out_d[sqt * 128:(sqt + 1) * 128, 512:1024],
                        o_sb[:, 512:1024])

            for sqt in range(NSQT):
                # out-proj of the in-flight sqt first: its copies enter the
                # DVE/ACT queues ahead of later normalizes, so the output
                # chain is never queued behind the norm sweep.
                if CFG.get("mmfirst", 1) and sqt >= CFG.get("skew", 2):
                    tail_mms(sqt - CFG.get("skew", 2))
                norm_one(1, 0, sqt, "dve")
                norm_one(1, 1, sqt, "act")
                ps_t = pp.tile([128, 128], BF, tag="pp", name="ps_t")
                nc.tensor.transpose(
                    ps_t[:, :], in_=ctx_sb[:, 1, sqt, :], identity=idn_sb[:, :])
                with nc.allow_low_precision(reason="bf16 storage"):
                    if CFG.get("tpool", 1):
                        nc.gpsimd.tensor_copy(
                            ctxT_sb[:, 1, sqt * 128:(sqt + 1) * 128],
                            ps_t[:, :])
                    elif sqt % 2 == 0:
                        nc.vector.tensor_copy(
                            ctxT_sb[:, 1, sqt * 128:(sqt + 1) * 128],
                            ps_t[:, :])
                    else:
                        nc.scalar.copy(
                            ctxT_sb[:, 1, sqt * 128:(sqt + 1) * 128],
                            ps_t[:, :])
                if not CFG.get("mmfirst", 1) and sqt >= CFG.get("skew", 3):
                    tail_mms(sqt - CFG.get("skew", 3))
            for s in range(NSQT - CFG.get("skew", 2 if CFG.get("mmfirst", 1)
                           else 3), NSQT):
                tail_mms(s)

        cpool_cm.__exit__(None, None, None)

    nc.compile()
    return nc


def get_nc(skp=SK):
    key = ("nc", skp)
    if key not in _CACHE:
        _CACHE[key] = _build(skp)
    return _CACHE[key]


def make_in_maps(query, key, value, key_mask, Wq, bq, Wk, bk, Wv, bv, Wo, bo):
    f32 = lambda x: np.asarray(x, dtype=np.float32)
    bf = lambda x: np.ascontiguousarray(np.asarray(x, dtype=np.float32),
                                        dtype=np.float32).astype(BF_NP)
    query, key, value = f32(query), f32(key), f32(value)
    Wq, bq, Wk, bk = f32(Wq), f32(bq), f32(Wk), f32(bk)
    Wv, Wo = f32(Wv), f32(Wo)
    key_mask = np.asarray(key_mask)

    # compact unmasked keys; pad to a common multiple of 128
    keep = [np.nonzero(key_mask[b] != 0)[0] for b in range(B)]
    skp = max(512, int(-(-max(len(k) for k in keep) // 128) * 128))
    skp = min(skp, SK)

    idn = np.eye(128, dtype=np.float32).astype(BF_NP)
    ones = np.ones((128, 1), np.float32).astype(BF_NP)
    qT, kT, vT, mb = [], [], [], []
    for b in range(B):
        n = len(keep[b])
        kc = np.zeros((skp, IN), np.float32)
        vc = np.zeros((skp, IN), np.float32)
        kc[:n] = key[b][keep[b]]
        vc[:n] = value[b][keep[b]]
        mbias = np.full(skp, -1e9, np.float32)
        mbias[:n] = 0.0
        qT.append(np.ascontiguousarray(query[b].T).astype(BF_NP))
        kT.append(np.ascontiguousarray(kc.T).astype(BF_NP))
        vT.append(np.ascontiguousarray(vc.T).astype(BF_NP))
        mb.append(np.ascontiguousarray(mbias.reshape(skp // 128, 128).T))

    in_maps = []
    for c in range(NCORES):
        b, g = c // 4, c % 4
        S = slice(DH * g, DH * (g + 1))
        bqk = np.stack([bq[S][0:128], bq[S][128:256],
                        bk[S][0:128], bk[S][128:256]], axis=1)
        in_maps.append({
            "qT": qT[b], "kT": kT[b], "vT": vT[b],
            "wq": np.ascontiguousarray(Wq[:, S]).astype(BF_NP),
            "wk": np.ascontiguousarray(Wk[:, S]).astype(BF_NP),
            "wv": np.ascontiguousarray(Wv[:, S]).astype(BF_NP),
            "wo": np.ascontiguousarray(Wo[S, :]).astype(BF_NP),
            "bqk": np.ascontiguousarray(bqk),
            "mb": mb[b], "ones": ones, "idn": idn,
        })
    return in_maps, skp


def run(in_maps, skp=SK, trace=False):
    from concourse.bass_utils import run_bass_kernel_spmd
    nc = get_nc(skp)
    res = run_bass_kernel_spmd(nc, in_maps, list(range(NCORES)), trace=trace)
    _CACHE["last_results"] = res
    return res


def kernel(query, key, value, key_mask, Wq, bq, Wk, bk, Wv, bv, Wo, bo):
    in_maps, skp = make_in_maps(query, key, value, key_mask,
                                Wq, bq, Wk, bk, Wv, bv, Wo, bo)

    def gather():
        res = run(in_maps, skp)
        out = np.zeros((B, SQ, SQ), np.float32)
        for c in range(NCORES):
            out[c // 4] += np.asarray(res.results[c]["out"], np.float32)
        return out

    out = gather()
    # first-execution sanity: retry once if the device returned garbage
    # (NaN/Inf or implausibly small magnitudes)
    if not np.isfinite(out).all() or np.abs(out).max() < 1e-3:
        out = gather()
    bo_eff = np.asarray(bo, np.float32) + f32v(bv) @ np.asarray(Wo, np.float32)
    out += bo_eff[None, None, :]
    return out


def f32v(x):
    return np.asarray(x, dtype=np.float32)
